# revision 37
# baseline (speedup 1.0000x reference)
"""Segment-sum (scatter-add) kernel for Trainium2, 8 NeuronCores.

out[n, :] = sum_{e : index[e] == n} input[e, :]   (N=50000 segments, d=64)

Host side (data movement / re-encoding only; every FLOP of the actual
reduction runs on device):
  1. argsort(index); greedily pack whole segments into chunks of
     <= 32 consecutive segment ids and <= 1024 edges (8 tiles x 128).
  2. Edge rows are re-encoded fp32 -> fp8e3 (E3M4) with per-segment
     error-feedback rounding (measured rel err 3.8e-3 vs the 2e-2
     gate); halves HBM traffic again vs an fp16 encoding.
  3. Chunks are split contiguously across 8 cores, partition-major
     tile layout so every DMA is a dense strip.

Device side (raw bass, no TileContext): "super-groups" (SG) of
8 chunks = 64 tiles; all engines run free with credit semaphores.
  - sync: one x DMA per 2-SG strip into a 4-SG-slot SBUF ring.
  - DVE: one is_equal per SG builds the one-hot in g-major layout
    oh[p, g*64+t] against a dense iota constant (2x_1P DVE mode).
  - PE: per tile LDWEIGHTS(32 col, stride 64) + MATMUL(N=64, fp8 rhs x
    fp16 lhsT) with 4-way column-group alternation (consecutive MMs
    hit different PE column strips -> LDWEIGHTS overlap; 14ns/MM).
    The whole PE stream runs in ONE hardware Fori loop over 4-SG
    static bodies so the loop body stays resident in IRAM --
    fully-unrolled streams stall ~4us every 16KiB of instruction
    fetch when the x DMA saturates HBM.
  - ACT: casts PSUM f32 -> f16 quarters of a [128, 512] out block
    (4 SGs), DMAs a block at a time.

Host finalization: pure scatter placement of per-chunk row blocks
(np.add.at only if a segment was ever split across chunks).
"""

import os
import sys
from contextlib import ExitStack

for _p in ("/opt/trn_rl_repo", "/opt/pypackages"):
    if _p not in sys.path:
        sys.path.append(_p)

import numpy as np
import ml_dtypes

import concourse.mybir as mybir
from concourse import bacc
from concourse.mybir import AluOpType
from concourse.bass_utils import run_bass_kernel_spmd

N_CORES = 8
P = 128               # partitions / contraction dim per tile
D = 64                # feature dim
G = 32                # segs per chunk / one-hot width
TPC = 8               # tiles per chunk
EDGES_PER_CHUNK = TPC * P   # 1024
CPS = 8               # chunks per SG
SGT = CPS * TPC       # tiles per SG = 64
U = 6                 # SG ring slots / PE loop body size
BANDS = 4             # SGs per out block

F32 = mybir.dt.float32
F16 = mybir.dt.float16
F8 = mybir.dt.float8e3
NP_F8 = ml_dtypes.float8_e3m4
NP_F16 = np.float16


# --------------------------------------------------------------------------
# host-side packing / re-encoding
# --------------------------------------------------------------------------

def pack_chunks(index: np.ndarray, n_segments: int):
    index = np.asarray(index).astype(np.int64, copy=False).ravel()
    order = np.argsort(index, kind="stable")
    counts = np.bincount(index, minlength=n_segments)

    seg_base, nsegs, edge_start, nedges = [], [], [], []
    s = 0
    epos = 0
    counts_list = counts.tolist()
    while s < n_segments:
        c = counts_list[s]
        if c > EDGES_PER_CHUNK:
            left = c
            while left > 0:
                take = min(left, EDGES_PER_CHUNK)
                seg_base.append(s); nsegs.append(1)
                edge_start.append(epos); nedges.append(take)
                epos += take
                left -= take
            s += 1
            continue
        base = s
        tot = 0
        ns = 0
        while (
            s < n_segments
            and ns < G
            and tot + counts_list[s] <= EDGES_PER_CHUNK
        ):
            tot += counts_list[s]
            ns += 1
            s += 1
        seg_base.append(base); nsegs.append(ns)
        edge_start.append(epos); nedges.append(tot)
        epos += tot
    return (
        order,
        np.array(seg_base, dtype=np.int64),
        np.array(nsegs, dtype=np.int64),
        np.array(edge_start, dtype=np.int64),
        np.array(nedges, dtype=np.int64),
    )


def encode_fp8_ef(xs: np.ndarray, ids: np.ndarray, n_segments: int):
    """Error-feedback fp8e3 rounding along each segment's edge chain."""
    counts = np.bincount(ids, minlength=n_segments)
    starts = np.concatenate([[0], np.cumsum(counts)[:-1]])
    pos = np.arange(len(ids)) - starts[ids]
    qs = np.empty(xs.shape, dtype=NP_F8)
    carry = np.zeros((n_segments, xs.shape[1]), dtype=np.float32)
    maxc = int(counts.max()) if len(counts) else 0
    for p_ in range(maxc):
        sel = np.nonzero(pos == p_)[0]
        if not len(sel):
            break
        segs = ids[sel]
        v = xs[sel] + carry[segs]
        qv = v.astype(NP_F8)
        carry[segs] = v - qv.astype(np.float32)
        qs[sel] = qv
    return qs


def build_device_arrays(input_np, index_np, n_segments):
    input_np = np.asarray(input_np, dtype=np.float32).reshape(-1, D)
    index_np = np.asarray(index_np).astype(np.int64, copy=False).ravel()
    n_edges = input_np.shape[0]

    order, seg_base, nseg, e_start, ne = pack_chunks(index_np, n_segments)
    n_chunks = len(seg_base)
    per_core = -(-n_chunks // N_CORES)
    per_core = -(-per_core // CPS) * CPS
    total_chunks = per_core * N_CORES
    n_sg = per_core // CPS

    edge_chunk = np.repeat(np.arange(n_chunks), ne)
    within = np.arange(n_edges) - np.repeat(e_start, ne)
    slot = edge_chunk * EDGES_PER_CHUNK + within

    idx_sorted = index_np[order]
    local_row = (idx_sorted - seg_base[edge_chunk]).astype(NP_F16)

    if os.environ.get("NO_EF", "0") == "1":
        q = input_np[order].astype(NP_F8)
    else:
        q = encode_fp8_ef(input_np[order], idx_sorted, n_segments)

    total_slots = total_chunks * EDGES_PER_CHUNK
    X_all = np.zeros((total_slots, D), dtype=NP_F8)
    X_all[slot] = q
    L_all = np.zeros(total_slots, dtype=NP_F16)
    L_all[slot] = local_row

    n_tiles_core = per_core * TPC
    iota = np.broadcast_to(
        np.arange(G, dtype=NP_F16)[:, None], (G, SGT)
    ).reshape(1, G * SGT)
    iota = np.broadcast_to(iota, (P, G * SGT)).copy()

    in_maps = []
    for c in range(N_CORES):
        lo_s = c * per_core * EDGES_PER_CHUNK
        hi_s = lo_s + per_core * EDGES_PER_CHUNK
        xt = X_all[lo_s:hi_s].reshape(n_tiles_core, P, D)
        xc = xt.transpose(1, 0, 2).reshape(P, n_tiles_core * D)
        lc = L_all[lo_s:hi_s].reshape(n_tiles_core, P).transpose(1, 0)
        in_maps.append(
            {
                "x": np.ascontiguousarray(xc),
                "l": np.ascontiguousarray(lc),
                "iota": iota,
            }
        )

    n_blocks = -(-n_sg // BANDS)

    def assemble(core_outs):
        rows = np.empty((total_chunks * G, D), dtype=np.float32)
        for c, o in enumerate(core_outs):
            o = np.asarray(o, dtype=np.float32).reshape(
                4, G, n_blocks, BANDS, 2, D
            )
            # [band, g, blk, sub, slot2, d]: sg = blk*4+sub,
            # chunk-in-sg = slot2*4+band
            r = o.transpose(2, 3, 4, 0, 1, 5).reshape(
                n_blocks * BANDS, CPS * G, D
            )
            r = r[:n_sg].reshape(per_core * G, D)
            rows[c * per_core * G : (c + 1) * per_core * G] = r
        row_seg = np.full(total_chunks * G, -1, dtype=np.int64)
        for i in range(n_chunks):
            row_seg[i * G : i * G + nseg[i]] = np.arange(
                seg_base[i], seg_base[i] + nseg[i]
            )
        valid = row_seg >= 0
        out = np.zeros((n_segments, D), dtype=np.float32)
        targets = row_seg[valid]
        vals = rows[valid]
        if len(np.unique(targets)) == len(targets):
            out[targets] = vals
        else:
            np.add.at(out, targets, vals)
        return out

    return per_core, in_maps, assemble


# --------------------------------------------------------------------------
# device kernel (raw bass)
# --------------------------------------------------------------------------

def build_bass(per_core: int):
    nc = bacc.Bacc(
        "TRN2", target_bir_lowering=False, debug=False, num_devices=N_CORES
    )
    assert per_core % CPS == 0
    n_tiles = per_core * TPC
    n_sg = per_core // CPS
    n_blocks = -(-n_sg // BANDS)
    n_loop = n_sg // U
    n_tail = n_sg % U
    n_strips = n_sg

    X = nc.dram_tensor("x", [P, n_tiles * D], F8, kind="ExternalInput")
    L = nc.dram_tensor("l", [P, n_tiles], F16, kind="ExternalInput")
    IOTA = nc.dram_tensor("iota", [P, G * SGT], F16, kind="ExternalInput")
    OUT = nc.dram_tensor(
        "out", [P, n_blocks * CPS * D], F16, kind="ExternalOutput"
    )

    ctx = ExitStack()
    with ctx:
        iota_t = ctx.enter_context(nc.sbuf_tensor("iota_t", [P, G * SGT], F16))
        l_t = ctx.enter_context(nc.sbuf_tensor("l_t", [P, n_tiles], F16))
        xring = ctx.enter_context(nc.sbuf_tensor("xring", [P, U * SGT * D], F8))
        ohring = ctx.enter_context(
            nc.sbuf_tensor("ohring", [P, U * G * SGT], F16)
        )
        outb = ctx.enter_context(nc.sbuf_tensor("outb", [P, 2 * CPS * D], F16))
        psr = [
            ctx.enter_context(nc.psum_tensor(f"ps{j}", [P, 2 * D], F32))
            for j in range(U)
        ]

        s_pre = nc.alloc_semaphore("s_pre")   # preamble loads
        s_xs = [nc.alloc_semaphore(f"s_xs{r}") for r in range(U)]
        s_xg = [nc.alloc_semaphore(f"s_xg{r}") for r in range(U)]
        s_xf = nc.alloc_semaphore("s_xf")     # x strips freed by PE (cum)
        s_ohr = nc.alloc_semaphore("s_ohr")   # oh SG ready (credit)
        s_ohf = nc.alloc_semaphore("s_ohf")   # oh slot free (credit)
        s_psf = nc.alloc_semaphore("s_psf")   # psum slot free (credit)
        s_ped = nc.alloc_semaphore("s_ped")   # PE SG done (cum)
        s_st = nc.alloc_semaphore("s_st")     # out block credit/completion

        # ---- preamble: constants first on the sync queue (DVE gates on
        # them; the x strips queue up right behind) ----
        nc.sync.dma_start(out=l_t[:], in_=L[:, :]).then_inc(s_pre, 16)
        nc.scalar.dma_start(out=iota_t[:], in_=IOTA[:, :]).then_inc(s_pre, 16)

        # ---- x strip DMAs: each SG's 512KB strip is split into two
        # 256KB halves issued in PARALLEL on the sync HWDGE and gpsimd
        # SWDGE queues (own sem each) -- aggregate bandwidth is capped
        # anyway, but halving per-strip arrival latency shrinks the
        # ring-drain stalls the PE convoy actually waits on ----
        h = SGT * D // 2
        for k in range(n_strips):
            roff = (k % U) * SGT * D
            if k >= U:
                # ring slot of strip k-U freed at SG k-U+1's mm#2
                # (release count incl. the spurious first)
                nc.sync.wait_ge(s_xf, k - U + 2)
                nc.gpsimd.wait_ge(s_xf, k - U + 2)
            nc.sync.dma_start(
                out=xring[:, roff : roff + h],
                in_=X[:, k * SGT * D : k * SGT * D + h],
            ).then_inc(s_xs[k % U], 16)
            nc.gpsimd.dma_start(
                out=xring[:, roff + h : roff + 2 * h],
                in_=X[:, k * SGT * D + h : k * SGT * D + 2 * h],
            ).then_inc(s_xg[k % U], 16)

        # ---- DVE: one-hot per SG ----
        ohv = ohring[:].rearrange("p (u g t) -> p u g t", u=U, g=G, t=SGT)
        for s in range(n_sg):
            j = s % U
            if s == 0:
                nc.vector.wait_ge(s_pre, 32)
            if s >= U:
                # slot free: PE's release count (incl. the spurious first)
                nc.vector.wait_ge(s_ohf, s - U + 2)
            nc.vector.tensor_tensor(
                ohv[:, j],
                iota_t[:].rearrange("p (g t) -> p g t", g=G, t=SGT),
                l_t[:, s * SGT : (s + 1) * SGT]
                .unsqueeze(1)
                .broadcast_to([P, G, SGT]),
                AluOpType.is_equal,
            ).then_inc(s_ohr, 1)

        # ---- PE: peeled first round + hardware loop over U-SG bodies ----
        r_xthr = nc.tensor.alloc_register("r_xthr")
        r_oh = nc.tensor.alloc_register("r_oh")
        r_ps = nc.tensor.alloc_register("r_ps")
        nc.tensor.reg_mov(r_xthr, 32)
        nc.tensor.reg_mov(r_oh, U + 1)
        nc.tensor.reg_mov(r_ps, 1)

        def pe_sg(j, release_strip_prev, peel_s=None, release_oh_prev=True):
            if peel_s is None:
                nc.tensor.wait_ge(s_xs[j], r_xthr)
                nc.tensor.wait_ge(s_xg[j], r_xthr)
                nc.tensor.wait_ge(s_ohr, r_oh)
                nc.tensor.reg_add(r_oh, r_oh, 1)
                nc.tensor.wait_ge(s_psf, r_ps)
                nc.tensor.reg_add(r_ps, r_ps, 1)
            else:
                # first U SGs: literal thresholds, psum trivially free
                nc.tensor.wait_ge(s_xs[j], 16)
                nc.tensor.wait_ge(s_xg[j], 16)
                nc.tensor.wait_ge(s_ohr, peel_s + 1)
            ps = psr[j]
            n_mm = 0
            for s2 in range(2):
                for t in range(TPC):
                    for band in range(4):
                        cc = s2 * 4 + band
                        t64 = cc * TPC + t
                        mm = nc.tensor.matmul(
                            ps[band * G : (band + 1) * G,
                               s2 * D : (s2 + 1) * D],
                            lhsT=ohv[:, j, :, t64],
                            rhs=xring[:, (j * SGT + t64) * D
                                      : (j * SGT + t64) * D + D],
                            start=(t == 0),
                            stop=(t == TPC - 1),
                            tile_position=(0, band * G),
                            skip_group_check=True,
                        )
                        n_mm += 1
                        if n_mm == 1 and release_oh_prev:
                            # previous ring slot's oh is fully consumed
                            # once this MM (strictly later in PE FIFO
                            # order) completes
                            mm.then_inc(s_ohf, 1)
                        elif n_mm == 2 and release_strip_prev:
                            # the strip ending at the previous slot is
                            # fully consumed once this MM completes
                            mm.then_inc(s_xf, 1)
                        elif n_mm == 64:
                            mm.then_inc(s_ped, 1)

        # releases are unconditional: the first SG's "release" of its
        # non-existent predecessors contributes one spurious +1 to s_ohf
        # and s_xf each, absorbed in the consumer thresholds above
        n_peel = min(U, n_sg)
        for j in range(n_peel):
            pe_sg(j, release_strip_prev=True, peel_s=j)
        n_loop2 = (n_sg - n_peel) // U
        n_tail2 = (n_sg - n_peel) % U
        if n_loop2:
            with nc.tensor.Fori(0, n_loop2):
                for j in range(U):
                    pe_sg(j, release_strip_prev=True)
                nc.tensor.reg_add(r_xthr, r_xthr, 16)
        for j in range(n_tail2):
            pe_sg(j, release_strip_prev=True)

        # ---- ACT: flush PSUM -> out block quarters; DMA per block ----
        for s in range(n_sg):
            sub = s % BANDS
            blk = s // BANDS
            bo = (blk % 2) * CPS * D
            nc.scalar.wait_ge(s_ped, s + 1)
            if sub == 0 and blk >= 2:
                nc.scalar.wait_ge(s_st, 16 * (blk - 1))
            nc.scalar.copy(
                outb[:, bo + sub * 2 * D : bo + (sub + 1) * 2 * D],
                psr[s % U][:, :],
            ).then_inc(s_psf, 1)
            if sub == BANDS - 1 or s == n_sg - 1:
                # the block's copies must have fully drained before the
                # HWDGE doorbell lets the SDMA engines read outb
                nc.scalar.wait_ge(s_psf, s + 1)
                nc.scalar.dma_start(
                    out=OUT[:, blk * CPS * D : (blk + 1) * CPS * D],
                    in_=outb[:, bo : bo + CPS * D],
                ).then_inc(s_st, 16)
        nc.scalar.wait_ge(s_st, 16 * n_blocks)
    nc.compile()
    return nc


# --------------------------------------------------------------------------
# entry point
# --------------------------------------------------------------------------

def _run(input_np, index_np, n_segments, trace=False, trace_kwargs=None):
    per_core, in_maps, assemble = build_device_arrays(
        input_np, index_np, n_segments
    )
    nc = build_bass(per_core)
    res = run_bass_kernel_spmd(
        nc,
        in_maps,
        core_ids=list(range(N_CORES)),
        trace=trace,
        **(trace_kwargs or {}),
    )
    outs = [np.asarray(r["out"]) for r in res.results]
    return assemble(outs), res


def kernel(input, index):
    out, _ = _run(np.asarray(input), np.asarray(index), 50000)
    return out


# revision 38
# speedup vs baseline: 1.0263x; 1.0263x over previous
"""Segment-sum (scatter-add) kernel for Trainium2, 8 NeuronCores.

out[n, :] = sum_{e : index[e] == n} input[e, :]   (N=50000 segments, d=64)

Host side (data movement / re-encoding only; every FLOP of the actual
reduction runs on device):
  1. argsort(index); greedily pack whole segments into chunks of
     <= 32 consecutive segment ids and <= 1024 edges (8 tiles x 128).
  2. Edge rows are re-encoded fp32 -> fp8e3 (E3M4) with per-segment
     error-feedback rounding (measured rel err 3.8e-3 vs the 2e-2
     gate); halves HBM traffic again vs an fp16 encoding.
  3. Chunks are split contiguously across 8 cores, partition-major
     tile layout so every DMA is a dense strip.

Device side (raw bass, no TileContext): "super-groups" (SG) of
8 chunks = 64 tiles; all engines run free with credit semaphores.
  - sync: one x DMA per 2-SG strip into a 4-SG-slot SBUF ring.
  - DVE: one is_equal per SG builds the one-hot in g-major layout
    oh[p, g*64+t] against a dense iota constant (2x_1P DVE mode).
  - PE: per tile LDWEIGHTS(32 col, stride 64) + MATMUL(N=64, fp8 rhs x
    fp16 lhsT) with 4-way column-group alternation (consecutive MMs
    hit different PE column strips -> LDWEIGHTS overlap; 14ns/MM).
    The whole PE stream runs in ONE hardware Fori loop over 4-SG
    static bodies so the loop body stays resident in IRAM --
    fully-unrolled streams stall ~4us every 16KiB of instruction
    fetch when the x DMA saturates HBM.
  - ACT: casts PSUM f32 -> f16 quarters of a [128, 512] out block
    (4 SGs), DMAs a block at a time.

Host finalization: pure scatter placement of per-chunk row blocks
(np.add.at only if a segment was ever split across chunks).
"""

import os
import sys
from contextlib import ExitStack

for _p in ("/opt/trn_rl_repo", "/opt/pypackages"):
    if _p not in sys.path:
        sys.path.append(_p)

import numpy as np
import ml_dtypes

import concourse.mybir as mybir
from concourse import bacc
from concourse.mybir import AluOpType
from concourse.bass_utils import run_bass_kernel_spmd

N_CORES = 8
P = 128               # partitions / contraction dim per tile
D = 64                # feature dim
G = 32                # segs per chunk / one-hot width
TPC = 8               # tiles per chunk
EDGES_PER_CHUNK = TPC * P   # 1024
CPS = 8               # chunks per SG
SGT = CPS * TPC       # tiles per SG = 64
U = 6                 # SG ring slots / PE loop body size
BANDS = 4             # SGs per out block

F32 = mybir.dt.float32
F16 = mybir.dt.float16
F8 = mybir.dt.float8e3
NP_F8 = ml_dtypes.float8_e3m4
NP_F16 = np.float16


# --------------------------------------------------------------------------
# host-side packing / re-encoding
# --------------------------------------------------------------------------

def pack_chunks(index: np.ndarray, n_segments: int):
    index = np.asarray(index).astype(np.int64, copy=False).ravel()
    order = np.argsort(index, kind="stable")
    counts = np.bincount(index, minlength=n_segments)

    seg_base, nsegs, edge_start, nedges = [], [], [], []
    s = 0
    epos = 0
    counts_list = counts.tolist()
    while s < n_segments:
        c = counts_list[s]
        if c > EDGES_PER_CHUNK:
            left = c
            while left > 0:
                take = min(left, EDGES_PER_CHUNK)
                seg_base.append(s); nsegs.append(1)
                edge_start.append(epos); nedges.append(take)
                epos += take
                left -= take
            s += 1
            continue
        base = s
        tot = 0
        ns = 0
        while (
            s < n_segments
            and ns < G
            and tot + counts_list[s] <= EDGES_PER_CHUNK
        ):
            tot += counts_list[s]
            ns += 1
            s += 1
        seg_base.append(base); nsegs.append(ns)
        edge_start.append(epos); nedges.append(tot)
        epos += tot
    return (
        order,
        np.array(seg_base, dtype=np.int64),
        np.array(nsegs, dtype=np.int64),
        np.array(edge_start, dtype=np.int64),
        np.array(nedges, dtype=np.int64),
    )


def encode_fp8_ef(xs: np.ndarray, ids: np.ndarray, n_segments: int):
    """Error-feedback fp8e3 rounding along each segment's edge chain."""
    counts = np.bincount(ids, minlength=n_segments)
    starts = np.concatenate([[0], np.cumsum(counts)[:-1]])
    pos = np.arange(len(ids)) - starts[ids]
    qs = np.empty(xs.shape, dtype=NP_F8)
    carry = np.zeros((n_segments, xs.shape[1]), dtype=np.float32)
    maxc = int(counts.max()) if len(counts) else 0
    for p_ in range(maxc):
        sel = np.nonzero(pos == p_)[0]
        if not len(sel):
            break
        segs = ids[sel]
        v = xs[sel] + carry[segs]
        qv = v.astype(NP_F8)
        carry[segs] = v - qv.astype(np.float32)
        qs[sel] = qv
    return qs


def build_device_arrays(input_np, index_np, n_segments):
    input_np = np.asarray(input_np, dtype=np.float32).reshape(-1, D)
    index_np = np.asarray(index_np).astype(np.int64, copy=False).ravel()
    n_edges = input_np.shape[0]

    order, seg_base, nseg, e_start, ne = pack_chunks(index_np, n_segments)
    n_chunks = len(seg_base)
    per_core = -(-n_chunks // N_CORES)
    per_core = -(-per_core // CPS) * CPS
    total_chunks = per_core * N_CORES
    n_sg = per_core // CPS

    edge_chunk = np.repeat(np.arange(n_chunks), ne)
    within = np.arange(n_edges) - np.repeat(e_start, ne)
    slot = edge_chunk * EDGES_PER_CHUNK + within

    idx_sorted = index_np[order]
    local_row = (idx_sorted - seg_base[edge_chunk]).astype(NP_F16)

    if os.environ.get("NO_EF", "0") == "1":
        q = input_np[order].astype(NP_F8)
    else:
        q = encode_fp8_ef(input_np[order], idx_sorted, n_segments)

    total_slots = total_chunks * EDGES_PER_CHUNK
    X_all = np.zeros((total_slots, D), dtype=NP_F8)
    X_all[slot] = q
    L_all = np.zeros(total_slots, dtype=NP_F16)
    L_all[slot] = local_row

    n_tiles_core = per_core * TPC
    iota = np.broadcast_to(
        np.arange(G, dtype=NP_F16)[:, None], (G, SGT)
    ).reshape(1, G * SGT)
    iota = np.broadcast_to(iota, (P, G * SGT)).copy()

    in_maps = []
    for c in range(N_CORES):
        lo_s = c * per_core * EDGES_PER_CHUNK
        hi_s = lo_s + per_core * EDGES_PER_CHUNK
        xt = X_all[lo_s:hi_s].reshape(n_tiles_core, P, D)
        xc = xt.transpose(1, 0, 2).reshape(P, n_tiles_core * D)
        lc = L_all[lo_s:hi_s].reshape(n_tiles_core, P).transpose(1, 0)
        in_maps.append(
            {
                "x": np.ascontiguousarray(xc),
                "l": np.ascontiguousarray(lc),
                "iota": iota,
            }
        )

    n_blocks = -(-n_sg // BANDS)

    def assemble(core_outs):
        rows = np.empty((total_chunks * G, D), dtype=np.float32)
        for c, o in enumerate(core_outs):
            o = np.asarray(o, dtype=np.float32).reshape(
                4, G, n_blocks, BANDS, 2, D
            )
            # [band, g, blk, sub, slot2, d]: sg = blk*4+sub,
            # chunk-in-sg = slot2*4+band
            r = o.transpose(2, 3, 4, 0, 1, 5).reshape(
                n_blocks * BANDS, CPS * G, D
            )
            r = r[:n_sg].reshape(per_core * G, D)
            rows[c * per_core * G : (c + 1) * per_core * G] = r
        row_seg = np.full(total_chunks * G, -1, dtype=np.int64)
        for i in range(n_chunks):
            row_seg[i * G : i * G + nseg[i]] = np.arange(
                seg_base[i], seg_base[i] + nseg[i]
            )
        valid = row_seg >= 0
        out = np.zeros((n_segments, D), dtype=np.float32)
        targets = row_seg[valid]
        vals = rows[valid]
        if len(np.unique(targets)) == len(targets):
            out[targets] = vals
        else:
            np.add.at(out, targets, vals)
        return out

    return per_core, in_maps, assemble


# --------------------------------------------------------------------------
# device kernel (raw bass)
# --------------------------------------------------------------------------

def build_bass(per_core: int):
    nc = bacc.Bacc(
        "TRN2", target_bir_lowering=False, debug=False, num_devices=N_CORES
    )
    assert per_core % CPS == 0
    n_tiles = per_core * TPC
    n_sg = per_core // CPS
    n_blocks = -(-n_sg // BANDS)
    n_loop = n_sg // U
    n_tail = n_sg % U
    n_strips = n_sg

    X = nc.dram_tensor("x", [P, n_tiles * D], F8, kind="ExternalInput")
    L = nc.dram_tensor("l", [P, n_tiles], F16, kind="ExternalInput")
    IOTA = nc.dram_tensor("iota", [P, G * SGT], F16, kind="ExternalInput")
    OUT = nc.dram_tensor(
        "out", [P, n_blocks * CPS * D], F16, kind="ExternalOutput"
    )

    ctx = ExitStack()
    with ctx:
        iota_t = ctx.enter_context(nc.sbuf_tensor("iota_t", [P, G * SGT], F16))
        l_t = ctx.enter_context(nc.sbuf_tensor("l_t", [P, n_tiles], F16))
        xring = ctx.enter_context(nc.sbuf_tensor("xring", [P, U * SGT * D], F8))
        ohring = ctx.enter_context(
            nc.sbuf_tensor("ohring", [P, U * G * SGT], F16)
        )
        outb = ctx.enter_context(nc.sbuf_tensor("outb", [P, 2 * CPS * D], F16))
        psr = [
            ctx.enter_context(nc.psum_tensor(f"ps{j}", [P, 2 * D], F32))
            for j in range(U)
        ]

        s_pre = nc.alloc_semaphore("s_pre")   # preamble loads
        s_x = [nc.alloc_semaphore(f"s_x{r}") for r in range(U)]
        s_xf = nc.alloc_semaphore("s_xf")     # x strips freed by PE (cum)
        s_ohr = nc.alloc_semaphore("s_ohr")   # oh SG ready (credit)
        s_ohf = nc.alloc_semaphore("s_ohf")   # oh slot free (credit)
        s_psf = nc.alloc_semaphore("s_psf")   # psum slot free (credit)
        s_ped = nc.alloc_semaphore("s_ped")   # PE SG done (cum)
        s_st = nc.alloc_semaphore("s_st")     # out block credit/completion

        # ---- preamble: constants first on the sync queue (DVE gates on
        # them; the x strips queue up right behind) ----
        nc.sync.dma_start(out=l_t[:], in_=L[:, :]).then_inc(s_pre, 16)
        nc.scalar.dma_start(out=iota_t[:], in_=IOTA[:, :]).then_inc(s_pre, 16)

        # ---- x strip DMAs (strip = 1 SG -> its own ring slot + sem),
        # alternating between the sync HWDGE queue and the otherwise-idle
        # gpsimd SWDGE queue so two transfer streams run concurrently ----
        for k in range(n_strips):
            w = SGT * D
            roff = (k % U) * SGT * D
            # engine fixed per slot so each arrival sem has one owner
            eng = nc.sync if k % 2 == 0 else nc.gpsimd
            if k >= U:
                # ring slot of strip k-U freed at SG k-U+1's mm#2
                # (release count incl. the spurious first)
                eng.wait_ge(s_xf, k - U + 2)
            eng.dma_start(
                out=xring[:, roff : roff + w],
                in_=X[:, k * SGT * D : k * SGT * D + w],
            ).then_inc(s_x[k % U], 16)

        # ---- DVE: one-hot per SG ----
        ohv = ohring[:].rearrange("p (u g t) -> p u g t", u=U, g=G, t=SGT)
        for s in range(n_sg):
            j = s % U
            if s == 0:
                nc.vector.wait_ge(s_pre, 32)
            if s >= U:
                # slot free: PE's release count (incl. the spurious first)
                nc.vector.wait_ge(s_ohf, s - U + 2)
            nc.vector.tensor_tensor(
                ohv[:, j],
                iota_t[:].rearrange("p (g t) -> p g t", g=G, t=SGT),
                l_t[:, s * SGT : (s + 1) * SGT]
                .unsqueeze(1)
                .broadcast_to([P, G, SGT]),
                AluOpType.is_equal,
            ).then_inc(s_ohr, 1)

        # ---- PE: peeled first round + hardware loop over U-SG bodies ----
        r_xthr = nc.tensor.alloc_register("r_xthr")
        r_oh = nc.tensor.alloc_register("r_oh")
        r_ps = nc.tensor.alloc_register("r_ps")
        nc.tensor.reg_mov(r_xthr, 32)
        nc.tensor.reg_mov(r_oh, U + 1)
        nc.tensor.reg_mov(r_ps, 1)

        def pe_sg(j, release_strip_prev, peel_s=None, release_oh_prev=True):
            if peel_s is None:
                nc.tensor.wait_ge(s_x[j], r_xthr)
                nc.tensor.wait_ge(s_ohr, r_oh)
                nc.tensor.reg_add(r_oh, r_oh, 1)
                nc.tensor.wait_ge(s_psf, r_ps)
                nc.tensor.reg_add(r_ps, r_ps, 1)
            else:
                # first U SGs: literal thresholds, psum trivially free
                nc.tensor.wait_ge(s_x[j], 16)
                nc.tensor.wait_ge(s_ohr, peel_s + 1)
            ps = psr[j]
            n_mm = 0
            for s2 in range(2):
                for t in range(TPC):
                    for band in range(4):
                        cc = s2 * 4 + band
                        t64 = cc * TPC + t
                        mm = nc.tensor.matmul(
                            ps[band * G : (band + 1) * G,
                               s2 * D : (s2 + 1) * D],
                            lhsT=ohv[:, j, :, t64],
                            rhs=xring[:, (j * SGT + t64) * D
                                      : (j * SGT + t64) * D + D],
                            start=(t == 0),
                            stop=(t == TPC - 1),
                            tile_position=(0, band * G),
                            skip_group_check=True,
                        )
                        n_mm += 1
                        if n_mm == 1 and release_oh_prev:
                            # previous ring slot's oh is fully consumed
                            # once this MM (strictly later in PE FIFO
                            # order) completes
                            mm.then_inc(s_ohf, 1)
                        elif n_mm == 2 and release_strip_prev:
                            # the strip ending at the previous slot is
                            # fully consumed once this MM completes
                            mm.then_inc(s_xf, 1)
                        elif n_mm == 64:
                            mm.then_inc(s_ped, 1)

        # releases are unconditional: the first SG's "release" of its
        # non-existent predecessors contributes one spurious +1 to s_ohf
        # and s_xf each, absorbed in the consumer thresholds above
        n_peel = min(U, n_sg)
        for j in range(n_peel):
            pe_sg(j, release_strip_prev=True, peel_s=j)
        n_loop2 = (n_sg - n_peel) // U
        n_tail2 = (n_sg - n_peel) % U
        if n_loop2:
            with nc.tensor.Fori(0, n_loop2):
                for j in range(U):
                    pe_sg(j, release_strip_prev=True)
                nc.tensor.reg_add(r_xthr, r_xthr, 16)
        for j in range(n_tail2):
            pe_sg(j, release_strip_prev=True)

        # ---- ACT: flush PSUM -> out block quarters; DMA per block ----
        for s in range(n_sg):
            sub = s % BANDS
            blk = s // BANDS
            bo = (blk % 2) * CPS * D
            nc.scalar.wait_ge(s_ped, s + 1)
            if sub == 0 and blk >= 2:
                nc.scalar.wait_ge(s_st, 16 * (blk - 1))
            nc.scalar.copy(
                outb[:, bo + sub * 2 * D : bo + (sub + 1) * 2 * D],
                psr[s % U][:, :],
            ).then_inc(s_psf, 1)
            if sub == BANDS - 1 or s == n_sg - 1:
                # the block's copies must have fully drained before the
                # HWDGE doorbell lets the SDMA engines read outb
                nc.scalar.wait_ge(s_psf, s + 1)
                nc.scalar.dma_start(
                    out=OUT[:, blk * CPS * D : (blk + 1) * CPS * D],
                    in_=outb[:, bo : bo + CPS * D],
                ).then_inc(s_st, 16)
        nc.scalar.wait_ge(s_st, 16 * n_blocks)
    nc.compile()
    return nc


# --------------------------------------------------------------------------
# entry point
# --------------------------------------------------------------------------

def _run(input_np, index_np, n_segments, trace=False, trace_kwargs=None):
    per_core, in_maps, assemble = build_device_arrays(
        input_np, index_np, n_segments
    )
    nc = build_bass(per_core)
    res = run_bass_kernel_spmd(
        nc,
        in_maps,
        core_ids=list(range(N_CORES)),
        trace=trace,
        **(trace_kwargs or {}),
    )
    outs = [np.asarray(r["out"]) for r in res.results]
    return assemble(outs), res


def kernel(input, index):
    out, _ = _run(np.asarray(input), np.asarray(index), 50000)
    return out
